# revision 2
# baseline (speedup 1.0000x reference)
"""DeformConv1d Trainium2 Bass kernel, v3.

Changes vs v2 baseline:
  - 3-pass conv_off: contraction packs (ci64 x tap-pair) into 128 partitions
    via xtap tiles (partitions 0-63 = x[group], 64-127 = same shifted +1),
    cutting conv matmuls from 5 to 3 per 128-out block.
  - The 6-term piecewise-linear interp sum is accumulated on the TENSOR
    engine (identity-lhsT matmuls into PSUM) instead of a DVE add tree.
  - Softmax denominator + reciprocal done on k-pair-packed psum [128,K*LT].
  - attn multiplies on Pool read the f32 reciprocal directly (no cast).
  - LT=256 with one-l-tile software pipelining across engines.
"""

import numpy as np

B, CIN, COUT, L, K, G = 8, 256, 256, 8192, 5, 4
PAD = 2
CPG = 64
MARG = 8
LT = 256
NLT = L // LT            # 32
NCH = 2
XW = L + 2 * MARG + 16   # 8224; extra tail for xtap phase overread
XCHK = 4 * LT            # xtap chunk covers 4 l-tiles
XTW = XCHK + 24          # chunk width incl conv window slop
NCHUNK = L // XCHK       # 8
KLT = K * LT             # 1280

_CACHE = {}
TRACE = False
LAST_EXEC_NS = None


def _pack_weights(w_off, b_off, weight, bias):
    f16 = np.float16
    w_r = w_off.reshape(2, CIN, K, CPG, K)  # [d, c, k, ci, kp]
    # 3-pass conv lhsT: block t = ((d*K + k)*NCH + ch)*3 + q
    w3 = np.zeros((128, 2 * K * NCH * 3, 128), f16)
    for d in range(2):
        for k in range(K):
            for ch in range(NCH):
                t0 = ((d * K + k) * NCH + ch) * 3
                sub = w_r[d, ch * 128:(ch + 1) * 128, k]  # [o128, ci64, kp5]
                for q in range(2):
                    blk = np.zeros((128, 128), np.float32)
                    blk[:64, :] = sub[:, :, 2 * q].T       # phase 0 -> tap 2q
                    blk[64:, :] = sub[:, :, 2 * q + 1].T   # phase 1 -> tap 2q+1
                    w3[:, t0 + q, :] = blk.astype(f16)
                blk = np.zeros((128, 128), np.float32)
                blk[:64, :] = sub[:, :, 4].T
                w3[:, t0 + 2, :] = blk.astype(f16)
    w_fr = weight.reshape(COUT, CPG, K)
    wfin = np.zeros((128, K * NCH, 128), f16)
    for k in range(K):
        for ch in range(NCH):
            blk = np.zeros((128, 128), np.float32)
            for half in range(2):
                g = ch * 2 + half
                sub = w_fr[g * 64:(g + 1) * 64, :, k]
                blk[half * 64:(half + 1) * 64, half * 64:(half + 1) * 64] = sub.T
            wfin[:, k * NCH + ch, :] = blk.astype(f16)
    b_r = b_off.reshape(2, CIN, K)
    boffs = np.zeros((128, NCH, 2 * K), np.float32)
    for ch in range(NCH):
        for d in range(2):
            for k in range(K):
                boffs[:, ch, d * K + k] = b_r[d, ch * 128:(ch + 1) * 128, k]
    bfin = bias.reshape(NCH, 128).T.astype(np.float32).copy()
    p = np.arange(128)
    ones_sm = (p[:, None] % 64 == p[None, :] % 64).astype(f16)
    ident = np.eye(128, dtype=f16)
    return tuple(np.ascontiguousarray(a) for a in
                 (w3, wfin, boffs, bfin, ones_sm, ident))


def _build(nc):
    import concourse.bass as bass
    import concourse.tile as tile
    import concourse.mybir as mybir
    from concourse.mybir import AluOpType as alu

    def ov(slice_ap, count0, count1):
        return bass.AP(tensor=slice_ap.tensor, offset=slice_ap.offset,
                       ap=[list(slice_ap.ap[0]), [1, count0], [1, count1]])

    def fv(anchor_ap, n):
        """Flat [128, n] view starting at anchor (contiguous elements)."""
        return bass.AP(tensor=anchor_ap.tensor, offset=anchor_ap.offset,
                       ap=[list(anchor_ap.ap[0]), [1, n]])

    f16 = mybir.dt.float16
    f32 = mybir.dt.float32
    AF = mybir.ActivationFunctionType

    x_d = nc.dram_tensor("x", [CIN, L], f32, kind="ExternalInput")
    w3_d = nc.dram_tensor("w3", [128, 2 * K * NCH * 3, 128], f16, kind="ExternalInput")
    wfin_d = nc.dram_tensor("wfin", [128, K * NCH, 128], f16, kind="ExternalInput")
    boffs_d = nc.dram_tensor("boffs", [128, NCH, 2 * K], f32, kind="ExternalInput")
    bfin_d = nc.dram_tensor("bfin", [128, NCH], f32, kind="ExternalInput")
    ones_d = nc.dram_tensor("ones_sm", [128, 128], f16, kind="ExternalInput")
    ident_d = nc.dram_tensor("ident", [128, 128], f16, kind="ExternalInput")
    out_d = nc.dram_tensor("out", [CIN, L], f32, kind="ExternalOutput")

    with tile.TileContext(nc) as tc:
        with (
            tc.tile_pool(name="consts", bufs=1) as consts,
            tc.tile_pool(name="resid", bufs=1) as resid,
            tc.tile_pool(name="stage", bufs=2) as stage,
            tc.tile_pool(name="xt", bufs=2) as xtp,
            tc.tile_pool(name="work", bufs=2) as work,
            tc.tile_pool(name="ps512", bufs=2, space="PSUM") as ps512,
            tc.tile_pool(name="pbig", bufs=2, space="PSUM") as pbig,
        ):
            w3_sb = consts.tile([128, 2 * K * NCH * 3, 128], f16, name="w3_sb", tag="w3_sb")
            nc.sync.dma_start(out=w3_sb, in_=w3_d[:, :, :])
            wfin_sb = consts.tile([128, K * NCH, 128], f16, name="wfin_sb", tag="wfin_sb")
            nc.sync.dma_start(out=wfin_sb, in_=wfin_d[:, :, :])
            boff_sb = consts.tile([128, NCH, 2 * K], f32, name="boff_sb", tag="boff_sb")
            nc.sync.dma_start(out=boff_sb, in_=boffs_d[:, :, :])
            bfin_sb = consts.tile([128, NCH], f32, name="bfin_sb", tag="bfin_sb")
            nc.sync.dma_start(out=bfin_sb, in_=bfin_d[:, :])
            ones_sb = consts.tile([128, 128], f16, name="ones_sb", tag="ones_sb")
            nc.sync.dma_start(out=ones_sb, in_=ones_d[:, :])
            id_sb = consts.tile([128, 128], f16, name="id_sb", tag="id_sb")
            nc.sync.dma_start(out=id_sb, in_=ident_d[:, :])

            # xpad: fp16 x with zero margins
            xpad = []
            for ch in range(NCH):
                xp = resid.tile([128, XW], f16, name=f"xpad{ch}", tag=f"xpad{ch}")
                nc.vector.memset(xp[:, 0:MARG], 0.0)
                nc.vector.memset(xp[:, MARG + L:XW], 0.0)
                xpad.append(xp)
            SST = 1024
            def emit_cast(i, ch):
                st = stage.tile([128, SST], f32, name="xstage", tag="xstage")
                nc.sync.dma_start(
                    out=st, in_=x_d[ch * 128:(ch + 1) * 128, i * SST:(i + 1) * SST])
                nc.scalar.activation(
                    out=xpad[ch][:, MARG + i * SST:MARG + (i + 1) * SST],
                    in_=st, func=AF.Copy, bias=0.0, scale=1.0)

            # xtap chunks: xt[r] partitions 0-63 = xpad[r//2][64*(r%2)+p, base+j],
            # partitions 64-127 same shifted +1.  xt col j <-> xpad col c*XCHK + j.
            def emit_xt_chunk(c):
                tiles = []
                for r in range(4):
                    xt = xtp.tile([128, XTW], f16, name=f"xt{r}", tag=f"xt{r}")
                    src = xpad[r // 2]
                    p0 = 64 * (r % 2)
                    nc.sync.dma_start(
                        out=xt[0:64, :],
                        in_=src[p0:p0 + 64, c * XCHK:c * XCHK + XTW])
                    nc.sync.dma_start(
                        out=xt[64:128, :],
                        in_=src[p0:p0 + 64, c * XCHK + 1:c * XCHK + 1 + XTW])
                    tiles.append(xt)
                return tiles

            for ch in range(NCH):
                for i in range(L // SST):
                    emit_cast(i, ch)
            xt_cur = emit_xt_chunk(0)
            xt_next = emit_xt_chunk(1)

            # software pipeline: iteration `it` emits
            #   A. final(lt-2) + og + out-dma        (PE/ACT/SP; deps long resolved)
            #   B. conv(lt) + off/exp drains          (PE/ACT)
            #   C. vsum(lt-1) + v16 drains            (PE/ACT)
            #   D. preps(lt)                          (Pool/DVE small)
            #   E. att(lt-1), y(lt-1)                 (Pool)
            #   F. den(lt)                            (PE)
            #   G. clamps+mults(lt)                   (DVE)
            #   H. recip(lt)                          (DVE)
            prev = None   # state of lt-1
            prev2 = None  # state of lt-2 (y + l0 for the final conv)

            for it in range(NLT + 2):
                lt = it
                # ---- A: final(lt-2)
                if prev2 is not None:
                    for ch in range(NCH):
                        pf = ps512.tile([128, 512], f32, name="pfin", tag="pconv")
                        y = prev2["y"][ch]
                        for k in range(K):
                            nc.tensor.matmul(
                                pf[:, 0:LT],
                                lhsT=wfin_sb[:, k * NCH + ch, :],
                                rhs=y[:, k, :],
                                start=(k == 0), stop=(k == K - 1))
                        og = work.tile([128, LT], f32, name="og", tag="og", bufs=2)
                        nc.scalar.activation(
                            out=og, in_=pf[:, 0:LT], func=AF.Identity,
                            bias=bfin_sb[:, ch:ch + 1], scale=1.0)
                        pl0 = prev2["l0"]
                        nc.sync.dma_start(
                            out=out_d[ch * 128:(ch + 1) * 128, pl0:pl0 + LT], in_=og)

                # ---- B: conv(lt) + drains
                if lt < NLT:
                    if lt % 4 == 0 and lt > 0:
                        c = lt // 4
                        xt_cur = xt_next
                        if c + 1 < NCHUNK:
                            xt_next = emit_xt_chunk(c + 1)
                    co = (lt % 4) * LT + MARG - 2

                    off_t, exp_t = [], []
                    for ch in range(NCH):
                        off_t.append(work.tile([128, K, LT], f16, name=f"off{ch}", tag=f"off{ch}", bufs=2))
                        exp_t.append(work.tile([128, K, LT], f16, name=f"exp{ch}", tag=f"exp{ch}", bufs=2))
                    for d in range(2):
                        for ch in range(NCH):
                            r = 2 * d + ch
                            for k in range(K):
                                t0 = ((d * K + k) * NCH + ch) * 3
                                ps = ps512.tile([128, 512], f32, name="pconv", tag="pconv")
                                for q in range(3):
                                    nc.tensor.matmul(
                                        ps[:, 0:LT],
                                        lhsT=w3_sb[:, t0 + q, :],
                                        rhs=xt_cur[r][:, co + 2 * q:co + 2 * q + LT],
                                        start=(q == 0), stop=(q == 2))
                                if d == 0:
                                    nc.scalar.activation(
                                        out=off_t[ch][:, k, :], in_=ps[:, 0:LT],
                                        func=AF.Identity,
                                        bias=boff_sb[:, ch, k:k + 1], scale=1.0)
                                else:
                                    nc.scalar.activation(
                                        out=exp_t[ch][:, k, :], in_=ps[:, 0:LT],
                                        func=AF.Exp,
                                        bias=boff_sb[:, ch, K + k:K + k + 1], scale=1.0)

                # ---- C: vsum(lt-1) + v16 drains
                if prev is not None:
                    v16 = []
                    for ch in range(NCH):
                        pv = pbig.tile([128, 1536], f32, name="pv", tag="pbig")
                        for jj in range(6):
                            mt = prev["m"][ch]
                            nc.tensor.matmul(pv[:, 0:512], lhsT=id_sb,
                                             rhs=fv(mt[:, jj, 0, 0:1], 512),
                                             start=(jj == 0), stop=False)
                            nc.tensor.matmul(pv[:, 512:1024], lhsT=id_sb,
                                             rhs=fv(mt[:, jj, 2, 0:1], 512),
                                             start=(jj == 0), stop=False)
                            nc.tensor.matmul(pv[:, 1024:1280], lhsT=id_sb,
                                             rhs=fv(mt[:, jj, 4, 0:1], 256),
                                             start=(jj == 0), stop=False)
                        bs2 = prev["bs2"][ch]
                        nc.tensor.matmul(pv[:, 0:512], lhsT=id_sb,
                                         rhs=ov(bs2[:, 0:1], 2, LT),
                                         start=False, stop=True)
                        nc.tensor.matmul(pv[:, 512:1024], lhsT=id_sb,
                                         rhs=ov(bs2[:, 2:3], 2, LT),
                                         start=False, stop=True)
                        nc.tensor.matmul(pv[:, 1024:1280], lhsT=id_sb,
                                         rhs=bs2[:, 4:4 + LT],
                                         start=False, stop=True)
                        v = work.tile([128, K, LT], f16, name=f"v{ch}", tag=f"v{ch}", bufs=1)
                        nc.scalar.activation(out=fv(v[:, 0, 0:1], 512), in_=pv[:, 0:512],
                                             func=AF.Copy, bias=0.0, scale=1.0)
                        nc.scalar.activation(out=fv(v[:, 2, 0:1], 512), in_=pv[:, 512:1024],
                                             func=AF.Copy, bias=0.0, scale=1.0)
                        nc.scalar.activation(out=fv(v[:, 4, 0:1], 256), in_=pv[:, 1024:1280],
                                             func=AF.Copy, bias=0.0, scale=1.0)
                        v16.append(v)

                # ---- D: preps(lt)
                if lt < NLT:
                    l0 = lt * LT
                    D1W = LT + 10
                    SPAN = LT + 4
                    d1_t, bs2_t = [], []
                    for ch in range(NCH):
                        d1 = work.tile([128, D1W], f16, name=f"d1_{ch}", tag=f"d1_{ch}", bufs=2)
                        m2 = MARG + l0 - 5
                        nc.gpsimd.tensor_tensor(
                            out=d1, in0=xpad[ch][:, m2 + 1:m2 + 1 + D1W],
                            in1=xpad[ch][:, m2:m2 + D1W], op=alu.subtract)
                        t1 = work.tile([128, SPAN], f16, name=f"t1_{ch}", tag=f"t1_{ch}", bufs=1)
                        nc.gpsimd.tensor_tensor(out=t1, in0=d1[:, 0:SPAN], in1=d1[:, 5:5 + SPAN], op=alu.subtract)
                        t2 = work.tile([128, SPAN], f16, name=f"t2_{ch}", tag=f"t2_{ch}", bufs=1)
                        nc.gpsimd.tensor_tensor(out=t2, in0=d1[:, 1:1 + SPAN], in1=d1[:, 4:4 + SPAN], op=alu.subtract)
                        bs = work.tile([128, SPAN], f16, name=f"bs_{ch}", tag=f"bs_{ch}", bufs=1)
                        nc.vector.scalar_tensor_tensor(
                            out=bs, in0=t1, scalar=2.0, in1=t2, op0=alu.mult, op1=alu.add)
                        bs2 = work.tile([128, SPAN], f16, name=f"bs2_{ch}", tag=f"bs2_{ch}", bufs=2)
                        nc.gpsimd.tensor_tensor(
                            out=bs2, in0=bs,
                            in1=xpad[ch][:, MARG + l0 - 2:MARG + l0 - 2 + SPAN], op=alu.add)
                        d1_t.append(d1)
                        bs2_t.append(bs2)

                # ---- E: att(lt-1), y(lt-1)
                if prev is not None:
                    y_t = []
                    for ch in range(NCH):
                        a = work.tile([128, K, LT], f16, name=f"att{ch}", tag=f"att{ch}", bufs=1)
                        nc.gpsimd.tensor_tensor(
                            out=a, in0=prev["exp"][ch], in1=prev["rc32"], op=alu.mult)
                        y = work.tile([128, K, LT], f16, name=f"y{ch}", tag=f"y{ch}", bufs=2)
                        nc.gpsimd.tensor_tensor(out=y, in0=v16[ch], in1=a, op=alu.mult)
                        y_t.append(y)
                    prev2 = {"y": y_t, "l0": prev["l0"]}
                else:
                    prev2 = None

                # ---- F: den(lt)
                if lt < NLT:
                    pd = pbig.tile([128, 1536], f32, name="pden", tag="pbig")
                    for p in range(2):
                        for ch in range(NCH):
                            nc.tensor.matmul(
                                pd[:, p * 512:(p + 1) * 512], lhsT=ones_sb,
                                rhs=fv(exp_t[ch][:, 2 * p, 0:1], 512),
                                start=(ch == 0), stop=(ch == 1))
                    for ch in range(NCH):
                        nc.tensor.matmul(
                            pd[:, 1024:1280], lhsT=ones_sb,
                            rhs=exp_t[ch][:, 4, :],
                            start=(ch == 0), stop=(ch == 1))

                    # ---- G: clamps + mults(lt)
                    m_t = []
                    for ch in range(NCH):
                        mt = work.tile([128, 6, K, LT], f16, name=f"m{ch}", tag=f"m{ch}", bufs=1)
                        u = work.tile([128, K, LT], f16, name=f"u{ch}", tag=f"u{ch}", bufs=1)
                        for jj, j in enumerate(range(-3, 3)):
                            nc.vector.tensor_scalar(
                                out=u, in0=off_t[ch],
                                scalar1=float(j), scalar2=float(j + 1),
                                op0=alu.max, op1=alu.min)
                            d1v = ov(d1_t[ch][:, j + 3:j + 4], K, LT)
                            nc.vector.tensor_tensor(
                                out=mt[:, jj], in0=u, in1=d1v, op=alu.mult)
                        m_t.append(mt)

                    # ---- H: recip(lt)
                    rc32 = work.tile([128, K, LT], f32, name="rc32", tag="rc32", bufs=1)
                    nc.vector.reciprocal_approx_fast(
                        out=fv(rc32[:, 0, 0:1], KLT), in_=pd[:, 0:KLT])

                    prev = {"exp": exp_t, "m": m_t, "bs2": bs2_t,
                            "rc32": rc32, "l0": l0}
                else:
                    prev = None
    return nc


def _get_compiled():
    if "nc" not in _CACHE:
        import concourse.bacc as bacc
        nc = bacc.Bacc()
        _build(nc)
        nc.compile()
        _CACHE["nc"] = nc
    return _CACHE["nc"]


def kernel(x, w_off, b_off, weight, bias):
    x = np.ascontiguousarray(np.asarray(x, dtype=np.float32))
    w_off = np.asarray(w_off, dtype=np.float32)
    b_off = np.asarray(b_off, dtype=np.float32)
    weight = np.asarray(weight, dtype=np.float32)
    bias = np.asarray(bias, dtype=np.float32)

    w3, wfin, boffs, bfin, ones_sm, ident = _pack_weights(w_off, b_off, weight, bias)
    nc = _get_compiled()

    from concourse.bass_utils import run_bass_kernel_spmd
    in_maps = []
    for b in range(B):
        in_maps.append({
            "x": np.ascontiguousarray(x[b]),
            "w3": w3,
            "wfin": wfin,
            "boffs": boffs,
            "bfin": bfin,
            "ones_sm": ones_sm,
            "ident": ident,
        })
    res = run_bass_kernel_spmd(nc, in_maps, core_ids=list(range(B)),
                               trace=TRACE, stitch_traces=TRACE)
    global LAST_EXEC_NS
    if res.exec_time_ns is not None:
        LAST_EXEC_NS = res.exec_time_ns
    out = np.stack([res.results[b]["out"] for b in range(B)], axis=0)
    return out
